# revision 23
# baseline (speedup 1.0000x reference)
"""Ewald real-space potential kernel for Trainium2 (8 NeuronCores, SPMD).

Computes pot = 0.5 * sum_{i != j} q_i * erf(d_ij / sqrt(2)) / d_ij * C over
all pairs of N=4096 atoms, C = 90.0474 / (2*pi).

v3 design — symmetry + single-pass elementwise chain, no diagonal mask:

1. Symmetric cover (1.78x less work than the full matrix): each core owns a
   512-column i-window and 20 j-blocks of 128 rows:
     blocks  0..11 : groups c+1, c+2, c+3 (full width, weight 1)
     blocks 12..15 : own group c (diagonal, weight 1/2)
     blocks 16..19 : antipodal group c+-4, quadrant-split half width
   Every unordered atom pair is covered exactly once (diag pairs twice at 1/2).

2. Elementwise chain is ONE ACT pass + ONE custom-DVE pass:
     w(d2) = C*erf(a*d)/d  with a = 1/sqrt(2).
   erf(x)/x is even in x => analytic in s = d2, so the near field is a plain
   polynomial in s. Far field (s > ~10) has erf == 1 => w = C/sqrt(s).
   Device computes  w' = min(P4(t), rinv')  where t = beta*s comes straight
   from the d2 matmul (coords pre-scaled by sqrt(beta)),
   P4(t) = t^4 + B3 t^3 + B2 t^2 + B1 t + 1 is the monic-normalized quartic
   minimax fit and rinv' = rsqrt(lam*t) = (C/c0)/sqrt(s) is a single
   Abs_reciprocal_sqrt activation (one table set, no erf/sqrt tables).
   The fit is constrained to stay above C/sqrt(s) past the crossover so
   min() performs the branch select. Global scale c0 folds into QI.

3. Self-pairs are NOT masked: the diagonal is biased to d2 = DIAG_EPS (the bias rides the r2j
   row, so it must stay small vs real pair distances), and the known
   contribution sum_i 0.5*q_i^2*c0*P4(beta*eps) is subtracted on the host.

4. PE: K=13 fp16 hi/lo-split matmul for t (lo*lo terms dropped) plus
   [128,1]-stationary reduce matmuls  s += qj^T @ w'  (pair weights folded
   into QJ). Two warm-up matmuls cover the input-DMA window; the
   quadrant-split anti super runs first so its short matmuls extend the
   warm-up and the tail ends on fine-grained 512-wide supers.
   Final stt computes pot = sum_i s_i * (c0*q_i); host sums the 8 partials
   and subtracts the diagonal term in float64.
"""

import numpy as np

P = 128
N = 4096
NCORES = 8
COLS = N // NCORES          # 512 i-columns per core
K = 13                      # split-matmul contraction rows
NBLK = 20                   # j-blocks per core (16 full + 4 half width)
SIGMA = 1.0
NORM_CONST = 90.0474 / (2.0 * np.pi)
DIAG_EPS = 0.01

# Constrained minimax quartic fit of C*erf(a*sqrt(s))/sqrt(s) on s in [0, 10]
# (LP: |Q-w|<=t on [0,10]; Q>=w on [10,16]; Q>=C/sqrt(s) on [16, 5200]).
QF = (1.14266144e+01, -1.85816289e+00, 2.41013009e-01,
      -1.81744163e-02, 5.76130940e-04)
BETA = float((QF[4] / QF[0]) ** 0.25)          # t = BETA * d2
B1 = float(QF[1] / QF[0] / BETA)
B2 = float(QF[2] / QF[0] / BETA ** 2)
B3 = float(QF[3] / QF[0] / BETA ** 3)
LAM = float((QF[0] / NORM_CONST) ** 2 / BETA)  # rsqrt(LAM*t) = (C/c0)/sqrt(s)


def _p4(t):
    return ((((t + B3) * t + B2) * t + B1) * t) + 1.0


# super-groups of j-blocks; each is 1024 elems wide in the free dim
# (the quadrant-split anti super leads: its short reduce matmuls double as
# tensor-engine warm-up and the final super's reduce tail stays regular)
SUPERS = (((16, 17, 18, 19),) + tuple((2 * i, 2 * i + 1) for i in range(7))
          + ((14,), ((15, 0, 256),), ((15, 256, 512),)))
# moving-operand column window per block (quadrant split for blocks 16..19)
BLK_CSLICE = [(0, 512)] * 16 + [(0, 256), (0, 256), (256, 512), (256, 512)]
N_WARM = 2                  # startup PE warm-up matmuls ([1,512] each)

_CACHE = {}


def _split2(v32):
    h = v32.astype(np.float16)
    l = (v32 - h.astype(np.float32)).astype(np.float16)
    return h, l


def _core_jblocks(c):
    """[(j_atom_start, weight, is_diag), ...] for the 20 blocks."""
    blocks = []
    for k in (1, 2, 3):
        g = (c + k) % NCORES
        blocks += [(g * COLS + b * P, 1.0, False) for b in range(4)]
    blocks += [(c * COLS + b * P, 0.5, True) for b in range(4)]
    g = (c + 4) % NCORES
    order = (0, 1, 2, 3) if c < 4 else (2, 3, 0, 1)
    blocks += [(g * COLS + b * P, 1.0, False) for b in order]
    return blocks


def _build_core_inputs(q, r):
    q = q.astype(np.float32)
    r = r.astype(np.float32)
    r2_64 = (r.astype(np.float64) ** 2).sum(1)
    sb = np.sqrt(BETA)

    in_maps = []
    for c in range(NCORES):
        win = slice(COLS * c, COLS * (c + 1))
        blocks = _core_jblocks(c)
        jidx = np.concatenate([np.arange(js, js + P) for js, _, _ in blocks])

        rows_j, rows_i = [], []
        for d in range(3):
            cj = (sb * r[jidx, d]).astype(np.float32)
            ui = (-2.0 * sb * r[win, d]).astype(np.float32)
            jh, jl = _split2(cj)
            ih, il = _split2(ui)
            rows_j += [jh, jh, jl]
            rows_i += [ih, il, ih]
        r2j = BETA * r2_64[jidx]
        for bi, (js, _, is_diag) in enumerate(blocks):
            if is_diag:
                r2j[bi * P:(bi + 1) * P] += BETA * DIAG_EPS
        jh, jl = _split2(r2j.astype(np.float32))
        ih, il = _split2((BETA * r2_64[win]).astype(np.float32))
        ones_j = np.ones(NBLK * P, np.float16)
        ones_i = np.ones(COLS, np.float16)
        rows_j += [jh, jl, ones_j, ones_j]
        rows_i += [ones_i, ones_i, ih, il]

        qj = np.stack([q[js:js + P] * w for js, w, _ in blocks], 1)
        qi_row = (q[win] * QF[0]).reshape(1, COLS).astype(np.float32)

        ajbi = np.concatenate([np.stack(rows_j), np.stack(rows_i)], 1)
        in_maps.append({
            "ajbi": ajbi.astype(np.float16),                    # [K, 2560+COLS]
            "qj": qj.astype(np.float16),                        # [P, NBLK]
            "qi": qi_row,                                       # [1, COLS]
        })
    return in_maps


def _diag_correction(q):
    """Exactly-known self-pair term the device sums: 0.5*q_i^2*c0*P4(b*eps)."""
    q64 = np.asarray(q, np.float64)
    qj16 = (0.5 * np.asarray(q, np.float32)).astype(np.float16)
    w_diag = _p4(BETA * DIAG_EPS)
    return float(np.sum(qj16.astype(np.float64) * q64) * QF[0] * w_diag)


def _register_qmin():
    """Register the quartic+min custom DVE op (row 17, v3) once."""
    import concourse.dve_ops as dvo
    from concourse.dve_spec import (
        C0, C1, C2, One, Spec, Src0, Src1, lower, minn,
    )
    from concourse.dve_uop import DveOpSpec

    name = "TENSOR_QUARTIC_MIN_EW"
    if name in dvo._SUB_OPCODE_FOR_NAME:
        return dvo.CUSTOM_DVE_OP_QMIN_EW

    t = Src0
    body = minn(((((t + C2) * t + C1) * t + C0) * t) + One, Src1)

    def ref(in0, in1, s0, s1, imm2):
        tt = in0.astype(np.float32)
        h = ((((tt + imm2) * tt + s1) * tt + s0) * tt) + np.float32(1.0)
        return np.minimum(h, in1.astype(np.float32)).astype(np.float32)

    spec = Spec(body=body, reference=ref)
    row = max(dvo._SUB_OPCODE_FOR_NAME.values()) + 1
    dvo._SUB_OPCODE_FOR_NAME[name] = row
    shas = {}
    for ver in ("v3",):
        uops = lower(spec, ver=ver)
        shas[ver] = DveOpSpec(name=name, opcode=row, uops=uops,
                              rd1_en=True).sha(ver)
    op = dvo.DveOp(name=name, spec=spec, subdim=False, uops_sha=shas)
    dvo.OPS.append(op)
    dvo.CUSTOM_DVE_SPECS[name] = spec
    dvo.CUSTOM_DVE_OP_QMIN_EW = op
    return op


def _build_program():
    import concourse.mybir as mybir
    import concourse.tile as tile
    from concourse import bacc

    qmin_op = _register_qmin()

    dt = mybir.dt
    nc = bacc.Bacc("TRN2", target_bir_lowering=False, debug=False,
                   num_devices=NCORES)

    ajbi = nc.dram_tensor("ajbi", [K, NBLK * P + COLS], dt.float16,
                          kind="ExternalInput")
    qj = nc.dram_tensor("qj", [P, NBLK], dt.float16, kind="ExternalInput")
    qi = nc.dram_tensor("qi", [1, COLS], dt.float32, kind="ExternalInput")
    pot = nc.dram_tensor("pot", [1, 1], dt.float32, kind="ExternalOutput")

    rsq_fn = mybir.ActivationFunctionType.Abs_reciprocal_sqrt
    SW = 2 * COLS  # super width

    with tile.TileContext(nc) as tc:
        with (
            tc.tile_pool(name="const", bufs=1) as cpool,
            tc.tile_pool(name="wts", bufs=4) as wpool,
            tc.tile_pool(name="rinvs", bufs=3) as rpool,
            tc.tile_pool(name="d2pool", bufs=3, space="PSUM") as ppool,
            tc.tile_pool(name="spool", bufs=1, space="PSUM") as spool,
        ):
            # PE warm-up: hold the tensor engine busy while the input DMAs
            # are in flight (writes cycle through the d2 PSUM pool, which
            # WAW-orders them before the real d2 supers).
            W0 = cpool.tile([P, COLS], dt.float16)
            nc.vector.memset(W0[:, :], 0.0)
            for _ in range(N_WARM):
                wtile = ppool.tile([P, SW], dt.float32, tag="d2")
                nc.tensor.matmul(wtile[0:1, :COLS], W0[:, 0:1], W0[:, :],
                                 start=True, stop=True)

            AJBI = cpool.tile([K, NBLK * P + COLS], dt.float16)
            nc.sync.dma_start(AJBI[:], ajbi[:])
            AJ = AJBI
            BI = AJBI[:, NBLK * P:]
            QJ = cpool.tile([P, NBLK], dt.float16)
            nc.sync.dma_start(QJ[:], qj[:])
            QI = cpool.tile([1, COLS], dt.float32)
            nc.sync.dma_start(QI[:], qi[:])

            s_ps = spool.tile([1, COLS], dt.float32)
            nc.vector.memset(s_ps[:, :], 0.0)

            w_tiles = []

            def emit_reduce(g):
                wt, offs = w_tiles[g]
                for (b, lo, hi), off in zip(bparts(SUPERS[g]), offs):
                    nc.tensor.matmul(s_ps[:, lo:hi], QJ[:, b:b + 1],
                                     wt[:, off:off + (hi - lo)],
                                     start=False,
                                     stop=(b == 15 and hi == 512),
                                     skip_group_check=True)

            def bparts(blks):
                return [(b, *BLK_CSLICE[b]) if isinstance(b, int) else b
                        for b in blks]

            for g, blks in enumerate(SUPERS):
                parts = bparts(blks)
                gw = sum(hi - lo for _, lo, hi in parts)
                d2 = ppool.tile([P, gw], dt.float32, tag="d2")
                off = 0
                offs = []
                for b, lo, hi in parts:
                    w = hi - lo
                    nc.tensor.matmul(
                        d2[:, off:off + w],
                        AJ[:, b * P:(b + 1) * P],
                        AJBI[:, NBLK * P + lo:NBLK * P + hi],
                        start=True, stop=True,
                    )
                    offs.append(off)
                    off += w
                rinv = rpool.tile([P, gw], dt.float32, tag="rinv")
                nc.scalar.activation(rinv[:, :], d2[:, :], rsq_fn,
                                     scale=float(LAM))
                wt = wpool.tile([P, gw], dt.float16, tag="wt")
                nc.vector._custom_dve(qmin_op, out=wt[:, :], in0=d2[:, :],
                                      in1=rinv[:, :], s0=float(B1),
                                      s1=float(B2), imm2=float(B3))
                w_tiles.append((wt, offs))
                if g >= 2:
                    emit_reduce(g - 2)
            emit_reduce(len(SUPERS) - 2)
            emit_reduce(len(SUPERS) - 1)

            sq = cpool.tile([1, COLS], dt.float32)
            pot_sb = cpool.tile([1, 1], dt.float32)
            nc.vector.scalar_tensor_tensor(
                out=sq[:, :], in0=s_ps[:, :], scalar=1.0, in1=QI[:, :],
                op0=mybir.AluOpType.mult, op1=mybir.AluOpType.mult,
                accum_out=pot_sb[:, :],
            )
            nc.sync.dma_start(pot[:, :], pot_sb[:, :])

    nc.compile()
    return nc


def _get_program():
    if "nc" not in _CACHE:
        _CACHE["nc"] = _build_program()
    return _CACHE["nc"]


def _run(q, r, trace=False, **trace_kwargs):
    from concourse.bass_utils import run_bass_kernel_spmd

    nc = _get_program()
    in_maps = _build_core_inputs(np.asarray(q), np.asarray(r))
    res = run_bass_kernel_spmd(nc, in_maps, core_ids=list(range(NCORES)),
                               trace=trace, **trace_kwargs)
    total = np.float64(0.0)
    for m in res.results:
        total += np.float64(m["pot"].reshape(-1).astype(np.float64).sum())
    total -= _diag_correction(q)
    return np.array([total], dtype=np.float32), res


def kernel(q, r, cell=None, batch=None):
    out, _ = _run(q, r, trace=False)
    return out
